# revision 1
# baseline (speedup 1.0000x reference)
"""Trainium2 Bass kernel for sparse (sliding-window, GQA, RoPE) attention.

Sharding: 8-way tensor-parallel over heads. Core c owns q-heads 4c..4c+3 and
kv-head c (wq/wk/wv column-parallel, wo row-parallel); each core produces a
full-shape partial output and the host sums the 8 partials (the all-reduce).

All matmuls run in float32r (full PE rate, ~1e-3 rel err). Scores are computed
transposed ([k, q]) so no on-device transposes are needed anywhere except a
cheap PE-transpose of V^T -> V. RoPE uses a host-side de-interleave permutation
of each head's dims (evens then odds) plus a sign-folded sin table, so the
rotation is two muls, a half-swap SBUF DMA, and one add. Softmax is max-free
(scores are small by construction) with normalization deferred to one
broadcast-multiply per (head, seq).
"""
import numpy as np
from contextlib import ExitStack

import concourse.bass as bass
from concourse import bacc
import concourse.mybir as mybir
import concourse.tile as tile
from concourse.bass_utils import run_bass_kernel_spmd

F32R = mybir.dt.float32r
F32 = mybir.dt.float32

NCORE = 8
T = 2048              # total tokens (2 seqs x 1024)
DIM = 4096
SEQ = 1024
NSEQ = 2
HD = 128              # head dim
NH = 4                # q heads per core
NKT = DIM // 128      # 32 contraction tiles
NTT = T // 128        # 16 token tiles
QB = 256              # attention q-block width
SCALE = float(HD) ** -0.5
KPASS = 8             # k-tiles per phase-1 pass
NPASS = NKT // KPASS  # 4

# per-(seq-local qb) score k-tile lists: (seq-local k-tile index, mask id)
# masks: -1 none, 0: j>=p (C0), 1: j>=p+128 (C1), 2: j<p (F0), 3: j<p+128 (F1)
QB_TILES = [
    [(0, 0), (1, 1)],
    [(0, -1), (1, -1), (2, 0), (3, 1)],
    [(0, 2), (1, 3), (2, -1), (3, -1), (4, 0), (5, 1)],
    [(2, 2), (3, 3), (4, -1), (5, -1), (6, 0), (7, 1)],
]
MAXKT = 6

_NC_CACHE = {}


def _build_nc(reps=1, internal_io=False, ablate="full"):
    nc = bacc.Bacc("TRN2", target_bir_lowering=False, debug=False,
                   num_devices=NCORE)
    if internal_io:
        # timing-only variant: big tensors live in device DRAM (no host
        # transfer per run); tiny dummy in/out keep the pjrt contract.
        def dram_in(name, shape, dt):
            return nc.dram_tensor(name, shape, dt).ap()
        dummy_in = nc.declare_dram_parameter("tin", [128, 128], F32, isOutput=False)
        dout = nc.declare_dram_parameter("tout", [128, 128], F32, isOutput=True)
        outp = nc.dram_tensor("outp_i", [T, DIM], F32).ap()
    else:
        def dram_in(name, shape, dt):
            return nc.declare_dram_parameter(name, shape, dt, isOutput=False)
        outp = nc.declare_dram_parameter("outp", [T, DIM], F32, isOutput=True)
    xT = dram_in("xT", [DIM, T], F32R)
    wqkvT = dram_in("wqkvT", [DIM, NH * HD + 2 * HD], F32R)
    woT = dram_in("woT", [NH * HD, DIM], F32R)
    cosT = dram_in("cosT", [128, T], F32R)
    sinT = dram_in("sinT", [128, T], F32R)
    aux = dram_in("aux", [128, 8 * QB + 129], F32R)

    with tile.TileContext(nc) as tc, ExitStack() as top:
        persist = top.enter_context(tc.tile_pool(name="persist", bufs=1))
        if internal_io:
            dtile = persist.tile([128, 128], F32, name="dtile", tag="dtile")
            nc.sync.dma_start(dtile[:], dummy_in[:])
            nc.sync.dma_start(dout[:], dtile[:])

        aux_sb = persist.tile([128, 8 * QB + 129], F32R, name="aux_sb", tag="aux")
        nc.sync.dma_start(aux_sb[:], aux[:])
        mask_sb = aux_sb[:, 0:8 * QB]
        ident = aux_sb[:, 8 * QB:8 * QB + 128]
        ones = aux_sb[:, 8 * QB + 128:8 * QB + 129]

        for _rep in range(reps):
         if ablate == "dmaout":
             with ExitStack() as phd:
                 pd = phd.enter_context(tc.tile_pool(name="pd", bufs=1))
                 t_ = pd.tile([128, T], F32, name="xw", tag="xw")
                 nc.gpsimd.memset(t_[:], 1.0)
                 for k in range(NKT):
                     nc.sync.dma_start(
                         outp[(k % 16) * 128:(k % 16) * 128 + 128, 0:T], t_[:])
             continue

         if ablate in ("dma", "dmain"):
             with ExitStack() as phd:
                 pd = phd.enter_context(tc.tile_pool(name="pd", bufs=1))
                 psd = phd.enter_context(tc.tile_pool(name="psd", bufs=1, space="PSUM"))
                 skd = psd.tile([1, 512], F32, name="skd", tag="skd")
                 for k in range(NKT):
                     t_ = pd.tile([128, T], F32R, name=f"xd{k}", tag="xd", bufs=4)
                     nc.sync.dma_start(t_[:], xT[k * 128:(k + 1) * 128, :])
                     if ablate == "dma":
                         nc.sync.dma_start(
                             outp[(k % 16) * 128:(k % 16) * 128 + 128, 0:T]
                             .bitcast(F32R),
                             t_[:])
                     else:
                         for q4 in range(4):
                             nc.tensor.matmul(
                                 skd[0:1, 0:512], ones,
                                 t_[:, q4 * 512:(q4 + 1) * 512],
                                 start=(k == 0 and q4 == 0),
                                 stop=(k == NKT - 1 and q4 == 3))
                 if ablate == "dmain":
                     skdc = persist.tile([1, 128], F32, name="skdc", tag="skdc")
                     nc.vector.tensor_copy(skdc[:], skd[0:1, 0:128])
                     nc.sync.dma_start(outp[0:1, 0:128], skdc[:])
             continue

         pq_es = ExitStack()
         pq = pq_es.enter_context(tc.tile_pool(name="pq", bufs=1))
         QTp = [pq.tile([128, 2 * T], F32R, name=f"QTp{p_}", tag=f"QTp{p_}")
                for p_ in range(NH // 2)]
         KT = pq.tile([128, T], F32R, name="KT", tag="KT")
         V = [pq.tile([128, HD], F32R, name=f"V{j}", tag=f"V{j}")
              for j in range(NTT)]
         # ---------------- phase 1: projections + rope -----------------
         with ExitStack() as ph1:
             p1 = ph1.enter_context(tc.tile_pool(name="p1", bufs=1))
             ps1 = ph1.enter_context(tc.tile_pool(name="ps1", bufs=6, space="PSUM"))

             cos_sb = p1.tile([128, T], F32R, name="cos_sb", tag="cos")
             sin_sb = p1.tile([128, T], F32R, name="sin_sb", tag="sin")
             nc.sync.dma_start(cos_sb[:], cosT[:])
             nc.sync.dma_start(sin_sb[:], sinT[:])
             VT = p1.tile([128, T], F32R, name="VT", tag="VT")

             def ld(src, k, c0, w, tag, bufs):
                 t_ = p1.tile([128, w], F32R, name=f"{tag}_{k}", tag=tag, bufs=bufs)
                 nc.sync.dma_start(t_[:], src[k * 128:(k + 1) * 128, c0:c0 + w])
                 return t_

             # (k-pass, t-half) subpasses; x tiles are half-T so the next
             # subpass's loads double-buffer within the pool window.
             for kp in range(NPASS):
                 w_t = [ld(wqkvT, kp * KPASS + k, 0, NH * HD + 2 * HD,
                           "wqs", KPASS + 5) for k in range(KPASS)]
                 for th in range(2):
                     xt = [ld(xT, kp * KPASS + k, th * 1024, 1024, "xs", KPASS + 6)
                           for k in range(KPASS)]
                     seqs = []
                     for ch in (2 * th, 2 * th + 1):
                         seqs.append((KT, None, ch, w_t,
                                      slice(NH * HD, NH * HD + HD)))
                         seqs.append((VT, None, ch, w_t,
                                      slice(NH * HD + HD, NH * HD + 2 * HD)))
                         for m in range(NH):
                             seqs.append((QTp[m // 2], m % 2, ch, w_t,
                                          slice(m * HD, (m + 1) * HD)))
                     for dest, e, ch, wts, msl in seqs:
                         acc = ps1.tile([128, 512], F32, name="acc", tag="acc")
                         xsl = slice((ch - 2 * th) * 512, (ch - 2 * th) * 512 + 512)
                         for k in range(KPASS):
                             nc.tensor.matmul(acc[:], wts[k][:, msl], xt[k][:, xsl],
                                              start=(k == 0), stop=(k == KPASS - 1))
                         if e is None:
                             dsl = dest[:, ch * 512:(ch + 1) * 512]
                         else:
                             # interleaved columns q*2+e of the pair tile
                             dsl = dest[:, ch * 1024 + e:(ch + 1) * 1024:2]
                         if kp == 0:
                             nc.scalar.copy(dsl, acc[:])
                         else:
                             nc.vector.tensor_add(dsl, dsl, acc[:])

             # V^T -> V natural via PE transpose
             for j in range(NTT):
                 vt_ps = ps1.tile([128, 128], F32R, name="vt_ps", tag="acc")
                 nc.tensor.transpose(vt_ps[:], VT[:, j * 128:(j + 1) * 128], ident)
                 nc.vector.tensor_copy(V[j][:], vt_ps[:])

             # RoPE on Q and K (in-place, chunked). sin_sb holds [+sin; -sin],
             # so rotated = ht*cos + swap_halves(ht*sin_sgn). Q pair tiles are
             # q-interleaved, so per-head views are stride-2 column APs.
             views = [(KT[:, ch * 512:(ch + 1) * 512], slice(ch * 512, (ch + 1) * 512))
                      for ch in range(4)]
             for p_ in range(NH // 2):
                 for e in range(2):
                     for ch in range(4):
                         views.append(
                             (QTp[p_][:, ch * 1024 + e:(ch + 1) * 1024:2],
                              slice(ch * 512, (ch + 1) * 512)))
             for hv, cs in views:
                 a = p1.tile([128, 512], F32R, name="ra", tag="ra", bufs=2)
                 b = p1.tile([128, 512], F32R, name="rb", tag="rb", bufs=2)
                 bs = p1.tile([128, 512], F32R, name="rbs", tag="rbs", bufs=2)
                 nc.vector.tensor_mul(a[:], hv, cos_sb[:, cs])
                 nc.vector.tensor_mul(b[:], hv, sin_sb[:, cs])
                 nc.sync.dma_start(bs[0:64, :], b[64:128, :])
                 nc.sync.dma_start(bs[64:128, :], b[0:64, :])
                 nc.vector.tensor_add(hv, a[:], bs[:])

         if ablate == "ph1":
             # consume QTp/KT/V so DCE cannot strip phase 1
             with tc.tile_pool(name="snk", bufs=1, space="PSUM") as snk:
                 sk = snk.tile([1, 128], F32, name="sk", tag="sk")
                 srcs = QTp + [KT] + V
                 for i, t_ in enumerate(srcs):
                     nc.tensor.matmul(sk[:], ones, t_[:, 0:128],
                                      start=(i == 0), stop=(i == len(srcs) - 1))
                 skc = persist.tile([1, 128], F32, name="skc", tag="skc")
                 nc.vector.tensor_copy(skc[:], sk[:])
                 nc.sync.dma_start(outp[0:1, 0:128], skc[:])
             pq_es.close()
             continue  # noqa

         # ---------------- phase 2+3: attention, output projection ------
         with ExitStack() as ph2:
             p2 = ph2.enter_context(tc.tile_pool(name="p2", bufs=1))

             # pair-interleaved attention outputs: columns q*2+e
             attnT = [p2.tile([128, 2 * T], F32R, name=f"AT{p_}", tag=f"AT{p_}")
                      for p_ in range(NH // 2)]
             gp_ctr = 0
             with ExitStack() as pha:
                 p2a = pha.enter_context(tc.tile_pool(name="p2a", bufs=1))
                 psA = pha.enter_context(tc.tile_pool(name="psA", bufs=1,
                                                      space="PSUM"))
                 for s in range(NSEQ):
                     for p_ in range(NH // 2):
                         lrow = p2a.tile([1, 2 * SEQ], F32, name="lrow",
                                        tag="lrow", bufs=2)
                         for qb in range(SEQ // QB):
                             tiles = QB_TILES[qb]
                             n = len(tiles)
                             qs2 = 2 * (s * SEQ + qb * QB)
                             qsl = slice(qs2, qs2 + 2 * QB)
                             pt = p2a.tile([128, MAXKT * 2 * QB], F32R, name="pt",
                                          tag="pt", bufs=4)
                             groups = [tiles[0:3], tiles[3:6]]
                             for gi, grp in enumerate(groups):
                                 if not grp:
                                     continue
                                 g0 = gi * 3
                                 sc = psA.tile([128, 3 * 2 * QB], F32, name="sc",
                                               tag="sc", bufs=2)
                                 for i, (j, _) in enumerate(grp):
                                     k0 = s * SEQ + j * 128
                                     nc.tensor.matmul(
                                         sc[:, i * 2 * QB:(i + 1) * 2 * QB],
                                         KT[:, k0:k0 + 128], QTp[p_][:, qsl],
                                         start=True, stop=True)
                                 w = len(grp) * 2 * QB
                                 nc.scalar.activation(
                                     pt[:, g0 * 2 * QB:g0 * 2 * QB + w],
                                     sc[:, 0:w],
                                     mybir.ActivationFunctionType.Exp,
                                     scale=SCALE)
                             for i, (j, mi) in enumerate(tiles):
                                 if mi < 0:
                                     continue
                                 psl_ = pt[:, i * 2 * QB:(i + 1) * 2 * QB]
                                 msl_ = mask_sb[:, mi * 2 * QB:(mi + 1) * 2 * QB]
                                 eng = nc.gpsimd if (gp_ctr % 3 == 2) else nc.vector
                                 gp_ctr += 1
                                 eng.tensor_mul(psl_, psl_, msl_)
                             ovlv = psA.tile([128, 1024], F32, name="ovlv",
                                             tag="ovlv", bufs=1)
                             ov = ovlv[:, 0:512]
                             lvv = ovlv[0:1, 512:1024]
                             for i, (j, _) in enumerate(tiles):
                                 nc.tensor.matmul(ov, V[s * 8 + j][:],
                                                  pt[:, i * 2 * QB:(i + 1) * 2 * QB],
                                                  start=(i == 0), stop=(i == n - 1))
                             for i, (j, _) in enumerate(tiles):
                                 nc.tensor.matmul(lvv, ones,
                                                  pt[:, i * 2 * QB:(i + 1) * 2 * QB],
                                                  start=(i == 0), stop=(i == n - 1))
                             nc.vector.tensor_copy(attnT[p_][:, qsl], ov)
                             nc.scalar.copy(lrow[0:1, qb * 2 * QB:(qb + 1) * 2 * QB],
                                            lvv)
                         li = p2a.tile([1, 2 * SEQ], F32, name="li", tag="li", bufs=2)
                         nc.vector.reciprocal(li[:], lrow[:])
                         lb = p2a.tile([128, 2 * SEQ], F32, name="lb", tag="lb",
                                      bufs=2)
                         nc.gpsimd.partition_broadcast(lb[:], li[:])
                         ssl = slice(2 * s * SEQ, 2 * (s + 1) * SEQ)
                         nc.vector.tensor_mul(attnT[p_][:, ssl],
                                              attnT[p_][:, ssl], lb[:])

             if ablate == "ph12":
                 with tc.tile_pool(name="snk2", bufs=1, space="PSUM") as snk2:
                     sk2 = snk2.tile([1, 128], F32, name="sk2", tag="sk2")
                     for i, t_ in enumerate(attnT):
                         nc.tensor.matmul(sk2[:], ones, t_[:, 0:128],
                                          start=(i == 0), stop=(i == 1))
                     skc2 = persist.tile([1, 128], F32, name="skc2", tag="skc2")
                     nc.vector.tensor_copy(skc2[:], sk2[:])
                     nc.sync.dma_start(outp[1:2, 0:128], skc2[:])

             # ---------------- phase 3: output projection -----------------
             if ablate == "ph12":
                 pass
             else:
              with ExitStack() as ph3:
                  p3 = ph3.enter_context(tc.tile_pool(name="p3", bufs=1))
                  ps3 = ph3.enter_context(tc.tile_pool(name="ps3", bufs=7,
                                                       space="PSUM"))
                  wo_t = [p3.tile([128, DIM], F32R, name=f"wo{h}", tag=f"wo{h}")
                          for h in range(NH)]
                  for h in range(NH):
                      nc.sync.dma_start(wo_t[h][:], woT[h * 128:(h + 1) * 128, :])
                  for tb in range(NTT):
                      tsl2 = [slice(tb * 256 + e, (tb + 1) * 256, 2)
                              for e in range(2)]
                      for sh in range(2):
                          stage = p3.tile([128, DIM // 2], F32, name="stage",
                                          tag="stage", bufs=4)
                          for cc in range(4):
                              chn = sh * 4 + cc
                              oc = ps3.tile([128, 512], F32, name="oc", tag="oc")
                              for h in range(NH):
                                  nc.tensor.matmul(
                                      oc[:], attnT[h // 2][:, tsl2[h % 2]],
                                      wo_t[h][:, chn * 512:(chn + 1) * 512],
                                      start=(h == 0), stop=(h == NH - 1))
                              c0 = cc * 512
                              nc.scalar.copy(stage[:, c0:c0 + 256], oc[:, 0:256])
                              nc.vector.tensor_copy(stage[:, c0 + 256:c0 + 512],
                                                    oc[:, 256:512])
                          nc.sync.dma_start(
                              outp[slice(tb * 128, (tb + 1) * 128),
                                   sh * (DIM // 2):(sh + 1) * (DIM // 2)],
                              stage[:])
         pq_es.close()

    nc.compile()
    return nc


def _get_nc():
    if "nc" not in _NC_CACHE:
        _NC_CACHE["nc"] = _build_nc()
    return _NC_CACHE["nc"]


def _host_prep(x, cos, sin, wq, wk, wv, wo):
    perm = np.concatenate([np.arange(0, 128, 2), np.arange(1, 128, 2)])
    wq_p = wq.reshape(32, 128, DIM)[:, perm, :].reshape(32 * 128, DIM)
    wk_p = wk.reshape(8, 128, DIM)[:, perm, :].reshape(8 * 128, DIM)
    xT = np.ascontiguousarray(x.T)
    cosT = np.ascontiguousarray(np.vstack([cos.T, cos.T]))
    sinT = np.ascontiguousarray(np.vstack([sin.T, -sin.T]))
    p = np.arange(128)[:, None]
    j = np.arange(QB)[None, :]
    masks = [(j >= p), (j >= p + 128), (j < p), (j < p + 128)]
    aux = np.concatenate(
        [np.repeat(m, 2, axis=1) for m in masks]
        + [np.eye(128, dtype=bool), np.ones((128, 1), dtype=bool)],
        axis=1).astype(np.float32)
    in_maps = []
    for c in range(NCORE):
        in_maps.append({
            "xT": xT,
            "wqkvT": np.ascontiguousarray(np.concatenate([
                wq_p[c * 512:(c + 1) * 512],
                wk_p[c * 128:(c + 1) * 128],
                wv[c * 128:(c + 1) * 128]], axis=0).T),
            "woT": np.ascontiguousarray(wo[:, c * 512:(c + 1) * 512].T),
            "cosT": cosT, "sinT": sinT, "aux": aux,
        })
    return in_maps


def kernel(x, cos, sin, wq, wk, wv, wo, n_seqs):
    x = np.asarray(x, dtype=np.float32)
    cos = np.asarray(cos, dtype=np.float32)
    sin = np.asarray(sin, dtype=np.float32)
    wq = np.asarray(wq, dtype=np.float32)
    wk = np.asarray(wk, dtype=np.float32)
    wv = np.asarray(wv, dtype=np.float32)
    wo = np.asarray(wo, dtype=np.float32)
    assert int(n_seqs) == NSEQ and x.shape == (T, DIM)

    nc = _get_nc()
    in_maps = _host_prep(x, cos, sin, wq, wk, wv, wo)
    res = run_bass_kernel_spmd(nc, in_maps, list(range(NCORE))).results
    out = np.zeros((T, DIM), dtype=np.float32)
    for c in range(NCORE):
        out += res[c]["outp"]
    return out



# revision 3
# speedup vs baseline: 1.4250x; 1.4250x over previous
"""Trainium2 Bass kernel v4 for sparse (sliding-window, GQA, RoPE) attention.

Sharding: 8-way tensor-parallel over heads. Core c owns q-heads 4c..4c+3 and
kv-head c (wq/wk/wv column-parallel, wo row-parallel); each core produces a
full-shape partial output (bf16) and the host sums the 8 partials.

All matmul operands are bf16 (same PE rate as f32r, half the DMA/SBUF, 2x
DVE); accumulation stays f32 in PSUM. Phase 1 accumulates the full 4096
contraction in PSUM, streaming x in quarter-T chunks with prefetch; RoPE is
two DVE muls + a half-swap SBUF DMA + an add (bf16 out). Phase 2 rotates
PSUM banks (sc2/ov3/lv1), uses reciprocal_approx_fast for the softmax
denominator and fuses normalization into the PSUM->attnT copy. Phase 3
(out-proj) is interleaved with phase 2 of the other sequence. Weights and
rope tables persist in SBUF across reps; reps>1 run under a For_i hardware
loop (constant program size for differential timing).
"""
import numpy as np
import ml_dtypes
from contextlib import ExitStack

import concourse.bass as bass
from concourse import bacc
import concourse.mybir as mybir
import concourse.tile as tile
from concourse.bass_utils import run_bass_kernel_spmd

F32 = mybir.dt.float32
BF16 = mybir.dt.bfloat16

NCORE = 8
T = 2048              # total tokens (2 seqs x 1024)
DIM = 4096
SEQ = 1024
NSEQ = 2
HD = 128              # head dim
NH = 4                # q heads per core
NKT = DIM // 128      # 32 contraction tiles
QB = 256              # attention q-block width
SCALE = float(HD) ** -0.5
NQT = 8               # phase-1 token chunks (256 tokens each)
QT = T // NQT         # 256
XS_BUFS = 48          # x-tile slots (one full chunk + half-chunk prefetch)

# per-(seq-local qb) score k-tile lists: (seq-local k-tile index, mask id)
# masks: -1 none, 0: j>=p (C0), 1: j>=p+128 (C1), 2: j<p (F0), 3: j<p+128 (F1)
QB_TILES = [
    [(0, 0), (1, 1)],
    [(0, -1), (1, -1), (2, 0), (3, 1)],
    [(0, 2), (1, 3), (2, -1), (3, -1), (4, 0), (5, 1)],
    [(2, 2), (3, 3), (4, -1), (5, -1), (6, 0), (7, 1)],
]

_NC_CACHE = {}


def _build_nc(reps=1, internal_io=False, ablate="full"):
    nc = bacc.Bacc("TRN2", target_bir_lowering=False, debug=False,
                   num_devices=NCORE)
    if internal_io:
        def dram_in(name, shape, dt):
            return nc.dram_tensor(name, shape, dt).ap()
        dummy_in = nc.declare_dram_parameter("tin", [128, 128], F32, isOutput=False)
        dout = nc.declare_dram_parameter("tout", [128, 128], F32, isOutput=True)
        outp = nc.dram_tensor("outp_i", [T, DIM], BF16).ap()
    else:
        def dram_in(name, shape, dt):
            return nc.declare_dram_parameter(name, shape, dt, isOutput=False)
        outp = nc.declare_dram_parameter("outp", [T, DIM], BF16, isOutput=True)
    xT = dram_in("xT", [DIM, T], BF16)
    wqkvT = dram_in("wqkvT", [DIM, NH * HD + 2 * HD], BF16)
    woT = dram_in("woT", [NH * HD, DIM], BF16)
    cosT = dram_in("cosT", [128, T], F32)
    sinT = dram_in("sinT", [128, T], F32)
    aux = dram_in("aux", [128, 8 * QB + 129], BF16)

    with tile.TileContext(nc) as tc, ExitStack() as top:
        persist = top.enter_context(tc.tile_pool(name="persist", bufs=1))
        if internal_io:
            dtile = persist.tile([128, 128], F32, name="dtile", tag="dtile")
            nc.sync.dma_start(dtile[:], dummy_in[:])
            nc.sync.dma_start(dout[:], dtile[:])

        aux_sb = persist.tile([128, 8 * QB + 129], BF16, name="aux_sb", tag="aux")
        nc.sync.dma_start(aux_sb[:], aux[:])
        mask_sb = aux_sb[:, 0:8 * QB]
        ident = aux_sb[:, 8 * QB:8 * QB + 128]
        ones = aux_sb[:, 8 * QB + 128:8 * QB + 129]

        ps_es = ExitStack()
        if ablate in ("dma", "dmain", "dmaout"):
            pD = ps_es.enter_context(tc.tile_pool(name="pD", bufs=1))
            psD = ps_es.enter_context(tc.tile_pool(name="psD", bufs=1,
                                                   space="PSUM"))

            def dbody():
                if ablate == "dmaout":
                    t_ = pD.tile([128, T], BF16, name="xw", tag="xw")
                    nc.gpsimd.memset(t_[:], 1.0)
                    for k in range(2 * 16):
                        nc.sync.dma_start(
                            outp[(k % 16) * 128:(k % 16) * 128 + 128,
                                 (k // 16) * 2048:(k // 16) * 2048 + 2048],
                            t_[:])
                    return
                skd = psD.tile([1, 512], F32, name="skd", tag="skd")
                for k in range(NKT):
                    t_ = pD.tile([128, T], BF16, name=f"xd{k}", tag="xd",
                                 bufs=4)
                    nc.sync.dma_start(t_[:], xT[k * 128:(k + 1) * 128, :])
                    if ablate == "dma":
                        nc.sync.dma_start(
                            outp[(k % 16) * 128:(k % 16) * 128 + 128, 0:T],
                            t_[:])
                    else:
                        for q4 in range(4):
                            nc.tensor.matmul(
                                skd[0:1, 0:512], ones,
                                t_[:, q4 * 512:(q4 + 1) * 512],
                                start=(k == 0 and q4 == 0),
                                stop=(k == NKT - 1 and q4 == 3))
                if ablate == "dmain":
                    skdc = pD.tile([1, 128], F32, name="skdc", tag="skdc")
                    nc.vector.tensor_copy(skdc[:], skd[0:1, 0:128])
                    nc.sync.dma_start(outp[0:1, 0:256].bitcast(F32), skdc[:])

            for _ in range(reps):
                dbody()
            ps_es.close()
            nc.compile()
            return nc

        # weights + rope tables persist across reps (loaded once)
        cos_sb = persist.tile([128, T], F32, name="cos_sb", tag="cos")
        sin_sb = persist.tile([128, T], F32, name="sin_sb", tag="sin")
        nc.sync.dma_start(cos_sb[:], cosT[:])
        nc.sync.dma_start(sin_sb[:], sinT[:])
        w_t = []
        for k in range(NKT):
            wt = persist.tile([128, NH * HD + 2 * HD], BF16, name=f"wq_{k}",
                              tag=f"wq_{k}")
            nc.sync.dma_start(wt[:], wqkvT[k * 128:(k + 1) * 128, :])
            w_t.append(wt)
        wo_t = [persist.tile([128, DIM], BF16, name=f"wo{h}", tag=f"wo{h}")
                for h in range(NH)]
        for h in range(NH):
            nc.sync.dma_start(wo_t[h][:], woT[h * 128:(h + 1) * 128, :])

        pq = ps_es.enter_context(tc.tile_pool(name="pq", bufs=1))
        p1 = ps_es.enter_context(tc.tile_pool(name="p1", bufs=1))
        p2 = ps_es.enter_context(tc.tile_pool(name="p2", bufs=1))
        psA = ps_es.enter_context(tc.tile_pool(name="psA", bufs=2, space="PSUM"))
        psO = ps_es.enter_context(tc.tile_pool(name="psO", bufs=3, space="PSUM"))
        psL = ps_es.enter_context(tc.tile_pool(name="psL", bufs=1, space="PSUM"))
        psC = ps_es.enter_context(tc.tile_pool(name="psC", bufs=2, space="PSUM"))

        # pair-interleaved Q (columns q*2+e), K, V natural
        QTp = [pq.tile([128, 2 * T], BF16, name=f"QTp{p_}", tag=f"QTp{p_}")
               for p_ in range(NH // 2)]
        KT = pq.tile([128, T], BF16, name="KT", tag="KT")
        V = [pq.tile([128, HD], BF16, name=f"V{j}", tag=f"V{j}")
             for j in range(T // 128)]
        VT = p1.tile([128, T], BF16, name="VT", tag="VT")
        attnT = [[p2.tile([128, 2 * SEQ], BF16, name=f"AT{s}_{p_}",
                          tag=f"AT{s}_{p_}") for p_ in range(NH // 2)]
                 for s in range(NSEQ)]

        def body():
            # ---------------- phase 1: projections + rope ---------------
            xs = {}

            def load_qt(qt):
                for k in range(NKT):
                    t_ = p1.tile([128, QT], BF16, name=f"x_{qt}_{k}",
                                 tag="xs", bufs=XS_BUFS)
                    nc.sync.dma_start(t_[:], xT[k * 128:(k + 1) * 128,
                                                qt * QT:(qt + 1) * QT])
                    xs[(qt, k)] = t_

            def rope_chunk(st, dest_ap, cs):
                # dest = st*cos + halfswap(st*sin_signed), written as bf16
                a = p1.tile([128, QT], F32, name="ra", tag="ra", bufs=2)
                b = p1.tile([128, QT], F32, name="rb", tag="rb", bufs=2)
                bs = p1.tile([128, QT], F32, name="rbs", tag="rbs", bufs=2)
                nc.vector.tensor_mul(a[:], st[:], cos_sb[:, cs])
                nc.vector.tensor_mul(b[:], st[:], sin_sb[:, cs])
                nc.sync.dma_start(bs[0:64, :], b[64:128, :])
                nc.sync.dma_start(bs[64:128, :], b[0:64, :])
                nc.vector.tensor_add(dest_ap, a[:], bs[:])

            load_qt(0)
            for qt in range(NQT):
                if qt + 1 < NQT:
                    load_qt(qt + 1)
                csl = slice(qt * QT, (qt + 1) * QT)
                # dim-blocks: 4 Q heads, then K, then V
                for db in range(6):
                    msl = slice(db * HD, (db + 1) * HD)
                    acc = psA.tile([128, QT], F32, name="acc", tag="acc")
                    for k in range(NKT):
                        nc.tensor.matmul(acc[:], w_t[k][:, msl],
                                         xs[(qt, k)][:],
                                         start=(k == 0), stop=(k == NKT - 1))
                    if db < 4:    # Q head -> rope -> interleaved pair store
                        p_, e = db // 2, db % 2
                        st = p1.tile([128, QT], F32, name="st", tag="st",
                                     bufs=4)
                        nc.scalar.copy(st[:], acc[:])
                        dsl = QTp[p_][:, qt * 2 * QT + e:(qt + 1) * 2 * QT:2]
                        rope_chunk(st, dsl, csl)
                    elif db == 4:  # K -> rope
                        st = p1.tile([128, QT], F32, name="st", tag="st",
                                     bufs=4)
                        nc.scalar.copy(st[:], acc[:])
                        rope_chunk(st, KT[:, csl], csl)
                    else:          # V -> bf16, transposed to natural below
                        nc.scalar.copy(VT[:, csl], acc[:])
                # V^T -> V natural via PE transpose for this chunk
                for jj in range(QT // 128):
                    j = qt * (QT // 128) + jj
                    vt_ps = psC.tile([128, 128], BF16, name="vt_ps",
                                     tag="oc")
                    nc.tensor.transpose(vt_ps[:],
                                        VT[:, j * 128:(j + 1) * 128], ident)
                    nc.vector.tensor_copy(V[j][:], vt_ps[:])

            if ablate == "ph1":
                sk = psL.tile([1, 128], F32, name="sk", tag="lv")
                srcs = QTp + [KT] + V
                for i, t_ in enumerate(srcs):
                    nc.tensor.matmul(sk[:], ones, t_[:, 0:128],
                                     start=(i == 0), stop=(i == len(srcs) - 1))
                skc = p1.tile([1, 128], F32, name="skc", tag="skc")
                nc.vector.tensor_copy(skc[:], sk[:])
                nc.sync.dma_start(outp[0:1, 0:256].bitcast(F32), skc[:])
                return

            # ---------------- phases 2+3 interleaved --------------------
            def attn_chunk(s, p_, qb):
                tiles = QB_TILES[qb]
                n = len(tiles)
                qs2 = 2 * (s * SEQ + qb * QB)
                qsl = slice(qs2, qs2 + 2 * QB)
                pts = []
                for i, (j, mi) in enumerate(tiles):
                    k0 = s * SEQ + j * 128
                    sc = psA.tile([128, 2 * QB], F32, name="sc", tag="acc")
                    nc.tensor.matmul(sc[:], KT[:, k0:k0 + 128],
                                     QTp[p_][:, qsl], start=True, stop=True)
                    pt = p2.tile([128, 2 * QB], BF16, name="pt", tag="pt",
                                 bufs=6)
                    nc.scalar.activation(pt[:], sc[:],
                                         mybir.ActivationFunctionType.Exp,
                                         scale=SCALE)
                    if mi >= 0:
                        msl_ = mask_sb[:, mi * 2 * QB:(mi + 1) * 2 * QB]
                        nc.vector.tensor_mul(pt[:], pt[:], msl_)
                    pts.append(pt)
                ov = psO.tile([128, 2 * QB], F32, name="ov", tag="ov")
                lv = psL.tile([1, 2 * QB], F32, name="lv", tag="lv")
                for i, (j, _) in enumerate(tiles):
                    nc.tensor.matmul(lv[:], ones, pts[i][:],
                                     start=(i == 0), stop=(i == n - 1))
                for i, (j, _) in enumerate(tiles):
                    nc.tensor.matmul(ov[:], V[s * 8 + j][:], pts[i][:],
                                     start=(i == 0), stop=(i == n - 1))
                li = p2.tile([1, 2 * QB], F32, name="li", tag="li", bufs=2)
                nc.vector.reciprocal_approx_fast(out=li[:], in_=lv[:])
                lb = p2.tile([128, 2 * QB], F32, name="lb", tag="lb", bufs=2)
                nc.gpsimd.partition_broadcast(lb[:], li[:])
                nc.vector.tensor_mul(
                    attnT[s][p_][:, qb * 2 * QB:(qb + 1) * 2 * QB],
                    ov[:], lb[:])

            def ph3_block(s, tbl):
                tb = s * 8 + tbl
                tsl2 = [slice(tbl * 256 + e, (tbl + 1) * 256, 2)
                        for e in range(2)]
                for sh in range(2):
                    stage = p2.tile([128, DIM // 2], BF16, name="stage",
                                    tag="stage", bufs=2)
                    for cc in range(4):
                        chn = sh * 4 + cc
                        oc = psC.tile([128, 512], F32, name="oc", tag="oc")
                        for h in range(NH):
                            nc.tensor.matmul(
                                oc[:], attnT[s][h // 2][:, tsl2[h % 2]],
                                wo_t[h][:, chn * 512:(chn + 1) * 512],
                                start=(h == 0), stop=(h == NH - 1))
                        c0 = cc * 512
                        nc.scalar.copy(stage[:, c0:c0 + 256], oc[:, 0:256])
                        nc.vector.tensor_copy(stage[:, c0 + 256:c0 + 512],
                                              oc[:, 256:512])
                    nc.sync.dma_start(
                        outp[slice(tb * 128, (tb + 1) * 128),
                             sh * (DIM // 2):(sh + 1) * (DIM // 2)],
                        stage[:])

            for p_ in range(NH // 2):
                for qb in range(SEQ // QB):
                    attn_chunk(0, p_, qb)
            if ablate == "ph12":
                for p_ in range(NH // 2):
                    for qb in range(SEQ // QB):
                        attn_chunk(1, p_, qb)
                sk2 = psL.tile([1, 128], F32, name="sk2", tag="lv")
                srcs = [attnT[s][p_] for s in range(2) for p_ in range(2)]
                for i, t_ in enumerate(srcs):
                    nc.tensor.matmul(sk2[:], ones, t_[:, 0:128],
                                     start=(i == 0), stop=(i == 3))
                skc2 = p2.tile([1, 128], F32, name="skc2", tag="skc2")
                nc.vector.tensor_copy(skc2[:], sk2[:])
                nc.sync.dma_start(outp[1:2, 0:256].bitcast(F32), skc2[:])
                return
            # interleave: ph3 of seq 0 with attention of seq 1
            s1_chunks = [(p_, qb) for p_ in range(NH // 2)
                         for qb in range(SEQ // QB)]
            for i in range(8):
                p_, qb = s1_chunks[i]
                attn_chunk(1, p_, qb)
                ph3_block(0, i)
            for tbl in range(8):
                ph3_block(1, tbl)

        for _ in range(reps):
            body()
        ps_es.close()

    nc.compile()
    return nc


def _get_nc():
    if "nc" not in _NC_CACHE:
        _NC_CACHE["nc"] = _build_nc()
    return _NC_CACHE["nc"]


def _host_prep(x, cos, sin, wq, wk, wv, wo):
    bf = ml_dtypes.bfloat16
    perm = np.concatenate([np.arange(0, 128, 2), np.arange(1, 128, 2)])
    wq_p = wq.reshape(32, 128, DIM)[:, perm, :].reshape(32 * 128, DIM)
    wk_p = wk.reshape(8, 128, DIM)[:, perm, :].reshape(8 * 128, DIM)
    xT = np.ascontiguousarray(x.T.astype(bf))
    cosT = np.ascontiguousarray(np.vstack([cos.T, cos.T]).astype(np.float32))
    sinT = np.ascontiguousarray(np.vstack([sin.T, -sin.T]).astype(np.float32))
    p = np.arange(128)[:, None]
    j = np.arange(QB)[None, :]
    masks = [(j >= p), (j >= p + 128), (j < p), (j < p + 128)]
    aux = np.concatenate(
        [np.repeat(m, 2, axis=1) for m in masks]
        + [np.eye(128, dtype=bool), np.ones((128, 1), dtype=bool)],
        axis=1).astype(bf)
    in_maps = []
    for c in range(NCORE):
        in_maps.append({
            "xT": xT,
            "wqkvT": np.ascontiguousarray(np.concatenate([
                wq_p[c * 512:(c + 1) * 512],
                wk_p[c * 128:(c + 1) * 128],
                wv[c * 128:(c + 1) * 128]], axis=0).T.astype(bf)),
            "woT": np.ascontiguousarray(wo[:, c * 512:(c + 1) * 512].T
                                        .astype(bf)),
            "cosT": cosT, "sinT": sinT, "aux": aux,
        })
    return in_maps


def kernel(x, cos, sin, wq, wk, wv, wo, n_seqs):
    x = np.asarray(x, dtype=np.float32)
    cos = np.asarray(cos, dtype=np.float32)
    sin = np.asarray(sin, dtype=np.float32)
    wq = np.asarray(wq, dtype=np.float32)
    wk = np.asarray(wk, dtype=np.float32)
    wv = np.asarray(wv, dtype=np.float32)
    wo = np.asarray(wo, dtype=np.float32)
    assert int(n_seqs) == NSEQ and x.shape == (T, DIM)

    nc = _get_nc()
    in_maps = _host_prep(x, cos, sin, wq, wk, wv, wo)
    res = run_bass_kernel_spmd(nc, in_maps, list(range(NCORE))).results
    out = np.zeros((T, DIM), dtype=np.float32)
    for c in range(NCORE):
        out += res[c]["outp"].astype(np.float32)
    return out
